# revision 20
# baseline (speedup 1.0000x reference)
"""Trainium2 Bass kernel for the Agent forward pass (3 MLPs + KDE mixture).

Device computes layers 0-2 of the three MLPs (encoder / policy / MDN) in
feature-major layout (fp16 matmul operands, fp32 psum) and ships the final
hidden activations; host does the three tiny layer-3 projections, the KDE
tail (25 components x 3 dims per row), and the global-gradient-norm mix,
which needs a cross-shard reduction anyway.

v3: the kernel is elementwise-bound (psum->sbuf relu at 1 elem/cycle/lane
on ACT+DVE; ~85us combined floor for 144 [128,1024] tiles). Structure:
 - 3-stage software pipeline over slot QUADS (L0 of quad q || L1 of q-1 ||
   L2 of q-2) so both relu engines stream continuously.
 - per-engine psum pools (ACT and DVE each own a 2-slot [128,1024]
   rotation; 8 banks total) so neither engine waits on the other's pace.
 - 8 consecutive same-weight matmuls per fill run (weight switch only
   every 8 MMs; pipelined MMs at ~225ns vs ~390ns isolated).
 - relu split ACT:DVE = 19:17 runs, matching measured 1124ns vs 1263ns
   per-op costs.

Self-contained: hardcodes all shapes; imports only numpy + concourse.
"""

import os

import numpy as np

import concourse.bacc as bacc
import concourse.mybir as mybir
import concourse.tile as tile
from concourse.bass_utils import run_bass_kernel_spmd

# Problem dims (hardcoded per spec)
B = 131072
NCORES = 8
BC = B // NCORES  # 16384 rows per core
NG, ADIM = 25, 3
H = 1.0
NI = 0.0005
KDE_C = float((2.0 * np.pi * H**ADIM) ** (-0.5))

NB = 1024  # batch columns per slot (= relu tile width)
NSLOTS = BC // NB  # 16
NMM = 512  # matmul moving-operand chunk
QS = 2  # slots per quad (same-weight matmul run = QS*2 chunks)
NQ = NSLOTS // QS  # 8
NBQ = QS * NB  # 2048 columns per quad

ACT_DT = mybir.dt.float16
ACT_NP = np.float16

# --- const pack column layout ---
_col = 0


def _take(n):
    global _col
    c = _col
    _col += n
    return c, _col


C_EW1 = _take(128)
C_EW2 = _take(128)
C_PW1 = _take(128)
C_PW2 = _take(128)
C_MW1 = _take(128)
C_MW2 = _take(128)
C_EW0 = _take(128)  # aug: rows 0-63 ew0, row 64 eb0
C_PW0 = _take(128)  # aug: rows 0-63 pw0[:64], row 64 pb0, rows 65-96 pw0[64:]
C_MW0 = _take(128)  # aug: row 64 mb0, rows 65-96 mw0
NCONST = _col

# f32 bias pack (per-partition bias vectors for relu ops)
B_EB1, B_MB1, B_PB1, B_EB2, B_MB2, B_PB2 = range(6)
NBIAS = 6


def _pack_consts(w):
    P = np.zeros((128, NCONST), ACT_NP)

    def put(cr, arr, r0=0):
        c0, c1 = cr
        a = np.asarray(arr, np.float32).astype(ACT_NP)
        P[r0 : r0 + a.shape[0], c0 : c0 + a.shape[1]] = a

    put(C_EW1, w["ew1"])
    put(C_EW2, w["ew2"])
    put(C_PW1, w["pw1"])
    put(C_PW2, w["pw2"])
    put(C_MW1, w["mw1"])
    put(C_MW2, w["mw2"])
    put(C_EW0, w["ew0"])
    put(C_EW0, w["eb0"][None, :], r0=64)
    put(C_PW0, w["pw0"][0:64])
    put(C_PW0, w["pb0"][None, :], r0=64)
    put(C_PW0, w["pw0"][64:96], r0=65)
    put(C_MW0, w["mb0"][None, :], r0=64)
    put(C_MW0, w["mw0"], r0=65)
    return P


def _pack_biases(w):
    Q = np.zeros((128, NBIAS), np.float32)
    for col, key in [(B_EB1, "eb1"), (B_MB1, "mb1"), (B_PB1, "pb1"),
                     (B_EB2, "eb2"), (B_MB2, "mb2"), (B_PB2, "pb2")]:
        Q[:, col] = np.asarray(w[key], np.float32)
    return Q


# Each same-weight run's QS psum tiles alternate ACT/DVE so both relu
# engines chew every run in parallel. A few runs go [A,A] to shift the
# overall unit split to 77:67 (ACT ~1124ns/op vs DVE ~1263ns/op).
_FLIP_RUNS = frozenset({5, 19, 33, 47, 61})


def _run_engines(ridx):
    if ridx in _FLIP_RUNS:
        return ("A",) * QS
    return tuple("A" if i % 2 == 0 else "V" for i in range(QS))


def build_program():
    """Build the per-core Bass program (same SPMD program on all 8 cores)."""
    nc = bacc.Bacc("TRN2", target_bir_lowering=False, debug=False)

    sg = nc.dram_tensor("sg", [128, BC], ACT_DT, kind="ExternalInput")
    wpack = nc.dram_tensor("wpack", [128, NCONST], ACT_DT, kind="ExternalInput")
    bpack = nc.dram_tensor("bpack", [128, NBIAS], mybir.dt.float32, kind="ExternalInput")
    out_e = nc.dram_tensor("out_e", [128, BC], ACT_DT, kind="ExternalOutput")
    out_m = nc.dram_tensor("out_m", [128, BC], ACT_DT, kind="ExternalOutput")
    out_p = nc.dram_tensor("out_p", [128, BC], ACT_DT, kind="ExternalOutput")

    relu = mybir.ActivationFunctionType.Relu
    add_op = mybir.AluOpType.add
    max_op = mybir.AluOpType.max

    with tile.TileContext(nc) as tc:
        with (
            tc.tile_pool(name="consts", bufs=1) as consts,
            tc.tile_pool(name="ins", bufs=2 * QS + 1) as ins,
            tc.tile_pool(name="acts", bufs=2) as acts,
            tc.tile_pool(name="outs", bufs=2) as outs,
            tc.tile_pool(name="psa", bufs=2, space="PSUM") as psa,
            tc.tile_pool(name="psv", bufs=2, space="PSUM") as psv,
        ):
            # Warm-up: a no-dep activation so the ~2.7us ACT_TABLE_LOAD for
            # the relu spline tables overlaps the input DMAs instead of
            # sitting on the critical path before the first real relu.
            warm = consts.tile([1, 16], mybir.dt.float32)
            nc.vector.memset(warm[:], 0.0)
            nc.scalar.activation(
                out=warm[:], in_=warm[:], func=mybir.ActivationFunctionType.Relu
            )

            # L0 weights (cols C_EW0..) land first so the first fills can
            # start as early as possible; the rest of the pack follows.
            W = consts.tile([128, NCONST], ACT_DT)
            c0 = C_EW0[0]
            nc.sync.dma_start(out=W[:, c0:NCONST], in_=wpack[:, c0:NCONST])
            BV = consts.tile([128, NBIAS], mybir.dt.float32)
            nc.sync.dma_start(out=BV[:], in_=bpack[:])
            nc.sync.dma_start(out=W[:, 0:c0], in_=wpack[:, 0:c0])

            in_tiles = {}

            def in_fetch(s):
                if s < NSLOTS and s not in in_tiles:
                    it = ins.tile([128, NB], ACT_DT, tag="in", name=f"in{s}")
                    nc.sync.dma_start(out=it[:], in_=sg[:, s * NB : (s + 1) * NB])
                    in_tiles[s] = it

            def wv(cr, r0=0, r1=128):
                c0, c1 = cr
                return W[r0:r1, c0:c1]

            _pn = [0]

            def psum(eng):
                _pn[0] += 1
                pool = psa if eng == "A" else psv
                return pool.tile(
                    [128, NB], mybir.dt.float32, tag=f"p{eng}", name=f"pp{_pn[0]}"
                )

            def relu_op(eng, out, in_, bcol=None):
                if eng == "A":
                    if bcol is None:
                        nc.scalar.activation(out=out, in_=in_, func=relu)
                    else:
                        nc.scalar.activation(
                            out=out, in_=in_, func=relu,
                            bias=BV[:, bcol : bcol + 1],
                        )
                else:
                    if bcol is None:
                        nc.vector.tensor_scalar_max(out=out, in0=in_, scalar1=0.0)
                    else:
                        nc.vector.tensor_scalar(
                            out=out, in0=in_,
                            scalar1=BV[:, bcol : bcol + 1], scalar2=0.0,
                            op0=add_op, op1=max_op,
                        )

            NETS = ("e", "m", "p")
            L0W = {"e": (C_EW0, 0, 65), "m": (C_MW0, 64, 97), "p": (C_PW0, 0, 97)}
            L1W = {"e": C_EW1, "m": C_MW1, "p": C_PW1}
            L2W = {"e": C_EW2, "m": C_MW2, "p": C_PW2}
            L1B = {"e": B_EB1, "m": B_MB1, "p": B_PB1}
            L2B = {"e": B_EB2, "m": B_MB2, "p": B_PB2}
            outd = {"e": out_e, "m": out_m, "p": out_p}

            a1 = {}  # (net, quad) -> [128, NBQ] tile
            a2 = {}
            _ridx = [0]

            def fill_run(engs, lhsT, rhs_of_slot):
                """QS*2 consecutive same-weight MMs across a quad's slots."""
                pps = []
                for i in range(QS):
                    pp = psum(engs[i])
                    rhs = rhs_of_slot(i)
                    for j in range(0, NB, NMM):
                        nc.tensor.matmul(
                            pp[:, j : j + NMM], lhsT, rhs[:, j : j + NMM],
                            start=True, stop=True,
                        )
                    pps.append(pp)
                return pps

            # 3-stage software pipeline over quads
            for s in range(QS + 1):
                in_fetch(s)
            for k in range(NQ + 2):
                for s in range(QS):
                    in_fetch((k + 1) * QS + s + 1)

                # stage A: layer-0 for quad k
                if k < NQ:
                    for net in NETS:
                        engs = _run_engines(_ridx[0]); _ridx[0] += 1
                        cr, r0, r1 = L0W[net]
                        pps = fill_run(
                            engs, wv(cr, r0, r1),
                            lambda i: in_tiles[k * QS + i][r0:r1, 0:NB],
                        )
                        t = acts.tile(
                            [128, NBQ], ACT_DT, tag=f"a1{net}", name=f"a1{net}{k}"
                        )
                        for i in range(QS):
                            relu_op(engs[i], t[:, i * NB : (i + 1) * NB], pps[i][:])
                        a1[(net, k)] = t

                # stage B: layer-1 for quad k-1
                q1 = k - 1
                if 0 <= q1 < NQ:
                    for net in NETS:
                        engs = _run_engines(_ridx[0]); _ridx[0] += 1
                        src = a1.pop((net, q1))
                        pps = fill_run(
                            engs, wv(L1W[net]),
                            lambda i: src[:, i * NB : (i + 1) * NB],
                        )
                        t = acts.tile(
                            [128, NBQ], ACT_DT, tag=f"a2{net}", name=f"a2{net}{q1}"
                        )
                        for i in range(QS):
                            relu_op(
                                engs[i], t[:, i * NB : (i + 1) * NB], pps[i][:],
                                L1B[net],
                            )
                        a2[(net, q1)] = t

                # stage C: layer-2 for quad k-2, then DMA out
                q2 = k - 2
                if 0 <= q2 < NQ:
                    for net in NETS:
                        engs = _run_engines(_ridx[0]); _ridx[0] += 1
                        src = a2.pop((net, q2))
                        pps = fill_run(
                            engs, wv(L2W[net]),
                            lambda i: src[:, i * NB : (i + 1) * NB],
                        )
                        t = outs.tile(
                            [128, NBQ], ACT_DT, tag=f"o{net}", name=f"o{net}{q2}"
                        )
                        for i in range(QS):
                            relu_op(
                                engs[i], t[:, i * NB : (i + 1) * NB], pps[i][:],
                                L2B[net],
                            )
                            if q2 >= NQ - 2:
                                # tail quads: per-slot DMA so the final
                                # drain after the last relu is small
                                s = q2 * QS + i
                                nc.sync.dma_start(
                                    out=outd[net][:, s * NB : (s + 1) * NB],
                                    in_=t[:, i * NB : (i + 1) * NB],
                                )
                        if q2 < NQ - 2:
                            nc.sync.dma_start(
                                out=outd[net][:, q2 * NBQ : (q2 + 1) * NBQ],
                                in_=t[:],
                            )

    nc.compile()
    return nc


_NC = None
LAST_RESULTS = None  # BassKernelResults from the most recent run (for test.py)


def _get_nc():
    global _NC
    if _NC is None:
        _NC = build_program()
    return _NC


def kernel(**inputs):
    global LAST_RESULTS
    w = {k: np.asarray(v, np.float32) for k, v in inputs.items()}
    s, g = w["s"], w["g"]

    wpack = _pack_consts(w)
    bpack = _pack_biases(w)
    in_maps = []
    for c in range(NCORES):
        r0 = c * BC
        sgT = np.zeros((128, BC), ACT_NP)
        sgT[0:64] = s[r0 : r0 + BC].T.astype(ACT_NP)
        sgT[64] = 1.0
        sgT[65:97] = g[r0 : r0 + BC].T.astype(ACT_NP)
        in_maps.append(
            {"sg": np.ascontiguousarray(sgT), "wpack": wpack, "bpack": bpack}
        )

    nc = _get_nc()
    res = run_bass_kernel_spmd(
        nc,
        in_maps,
        core_ids=list(range(NCORES)),
        trace=bool(int(os.environ.get("KERNEL_TRACE", "0"))),
    )
    LAST_RESULTS = res

    a3e = np.empty((B, 128), np.float32)
    a3m = np.empty((B, 128), np.float32)
    a3p = np.empty((B, 128), np.float32)
    for c in range(NCORES):
        r0 = c * BC
        a3e[r0 : r0 + BC] = res.results[c]["out_e"].T
        a3m[r0 : r0 + BC] = res.results[c]["out_m"].T
        a3p[r0 : r0 + BC] = res.results[c]["out_p"].T

    # ---- host layer-3 projections ----
    z = a3e @ w["ew3"] + w["eb3"]
    mu = a3m @ w["mw3"] + w["mb3"]
    ail = a3p @ w["pw3"] + w["pb3"]

    # ---- host KDE tail + global-norm mix ----
    diff = z[:, None, :] - mu.reshape(B, NG, ADIM)  # [B, 25, 3]
    delta = -0.5 * np.einsum("bnd,bnd->bn", diff, diff) / (H * H)
    p = KDE_C * np.exp(delta)  # [B, 25]
    rho = p.sum(axis=-1)  # [B]
    grad = -np.einsum("bn,bnd->bd", p, diff) / (H * H)
    grad = np.nan_to_num(grad, nan=0.0)
    gnorm = np.linalg.norm(grad)
    gradn = grad / gnorm * NI
    pm = np.tanh(rho * 0.002)[:, None]
    out = pm * ail + (1.0 - pm) * gradn
    return out.astype(np.float32)


# revision 25
# speedup vs baseline: 1.0163x; 1.0163x over previous
"""Trainium2 Bass kernel for the Agent forward pass (3 MLPs + KDE mixture).

Device computes layers 0-2 of the three MLPs (encoder / policy / MDN) in
feature-major layout (fp16 matmul operands, fp32 psum) and ships the final
hidden activations; host does the three tiny layer-3 projections, the KDE
tail (25 components x 3 dims per row), and the global-gradient-norm mix,
which needs a cross-shard reduction anyway.

v3: the kernel is elementwise-bound (psum->sbuf relu at 1 elem/cycle/lane
on ACT+DVE; ~85us combined floor for 144 [128,1024] tiles). Structure:
 - 3-stage software pipeline over slot QUADS (L0 of quad q || L1 of q-1 ||
   L2 of q-2) so both relu engines stream continuously.
 - per-engine psum pools (ACT and DVE each own a 2-slot [128,1024]
   rotation; 8 banks total) so neither engine waits on the other's pace.
 - 8 consecutive same-weight matmuls per fill run (weight switch only
   every 8 MMs; pipelined MMs at ~225ns vs ~390ns isolated).
 - relu split ACT:DVE = 19:17 runs, matching measured 1124ns vs 1263ns
   per-op costs.

Self-contained: hardcodes all shapes; imports only numpy + concourse.
"""

import os

import numpy as np

import concourse.bacc as bacc
import concourse.mybir as mybir
import concourse.tile as tile
from concourse.bass_utils import run_bass_kernel_spmd

# Problem dims (hardcoded per spec)
B = 131072
NCORES = 8
BC = B // NCORES  # 16384 rows per core
NG, ADIM = 25, 3
H = 1.0
NI = 0.0005
KDE_C = float((2.0 * np.pi * H**ADIM) ** (-0.5))

NB = 1024  # batch columns per slot (= relu tile width)
NSLOTS = BC // NB  # 16
NMM = 512  # matmul moving-operand chunk
QS = 2  # slots per quad (same-weight matmul run = QS*2 chunks)
NQ = NSLOTS // QS  # 8
NBQ = QS * NB  # 2048 columns per quad

ACT_DT = mybir.dt.float16
ACT_NP = np.float16

# --- const pack column layout ---
_col = 0


def _take(n):
    global _col
    c = _col
    _col += n
    return c, _col


C_EW1 = _take(128)
C_EW2 = _take(128)
C_PW1 = _take(128)
C_PW2 = _take(128)
C_MW1 = _take(128)
C_MW2 = _take(128)
C_EW0 = _take(128)  # aug: rows 0-63 ew0, row 64 eb0
C_PW0 = _take(128)  # aug: rows 0-63 pw0[:64], row 64 pb0, rows 65-96 pw0[64:]
C_MW0 = _take(128)  # aug: row 64 mb0, rows 65-96 mw0
NCONST = _col

# f32 bias pack (per-partition bias vectors for relu ops)
B_EB1, B_MB1, B_PB1, B_EB2, B_MB2, B_PB2 = range(6)
NBIAS = 6


def _pack_consts(w):
    P = np.zeros((128, NCONST), ACT_NP)

    def put(cr, arr, r0=0):
        c0, c1 = cr
        a = np.asarray(arr, np.float32).astype(ACT_NP)
        P[r0 : r0 + a.shape[0], c0 : c0 + a.shape[1]] = a

    put(C_EW1, w["ew1"])
    put(C_EW2, w["ew2"])
    put(C_PW1, w["pw1"])
    put(C_PW2, w["pw2"])
    put(C_MW1, w["mw1"])
    put(C_MW2, w["mw2"])
    put(C_EW0, w["ew0"])
    put(C_EW0, w["eb0"][None, :], r0=64)
    put(C_PW0, w["pw0"][0:64])
    put(C_PW0, w["pb0"][None, :], r0=64)
    put(C_PW0, w["pw0"][64:96], r0=65)
    put(C_MW0, w["mb0"][None, :], r0=64)
    put(C_MW0, w["mw0"], r0=65)
    return P


def _pack_biases(w):
    Q = np.zeros((128, NBIAS), np.float32)
    for col, key in [(B_EB1, "eb1"), (B_MB1, "mb1"), (B_PB1, "pb1"),
                     (B_EB2, "eb2"), (B_MB2, "mb2"), (B_PB2, "pb2")]:
        Q[:, col] = np.asarray(w[key], np.float32)
    return Q


# Each same-weight run's QS psum tiles alternate ACT/DVE so both relu
# engines chew every run in parallel. A few runs go [A,A] to shift the
# overall unit split to 77:67 (ACT ~1124ns/op vs DVE ~1263ns/op).
_FLIP_RUNS = frozenset({5, 19, 33, 47, 61})


def _run_engines(ridx):
    if ridx in _FLIP_RUNS:
        return ("A",) * QS
    return tuple("A" if i % 2 == 0 else "V" for i in range(QS))


def build_program():
    """Build the per-core Bass program (same SPMD program on all 8 cores)."""
    nc = bacc.Bacc("TRN2", target_bir_lowering=False, debug=False)

    sg = nc.dram_tensor("sg", [128, BC], ACT_DT, kind="ExternalInput")
    wpack = nc.dram_tensor("wpack", [128, NCONST], ACT_DT, kind="ExternalInput")
    bpack = nc.dram_tensor("bpack", [128, NBIAS], mybir.dt.float32, kind="ExternalInput")
    out_e = nc.dram_tensor("out_e", [128, BC], ACT_DT, kind="ExternalOutput")
    out_m = nc.dram_tensor("out_m", [128, BC], ACT_DT, kind="ExternalOutput")
    out_p = nc.dram_tensor("out_p", [128, BC], ACT_DT, kind="ExternalOutput")

    relu = mybir.ActivationFunctionType.Relu
    add_op = mybir.AluOpType.add
    max_op = mybir.AluOpType.max

    with tile.TileContext(nc) as tc:
        with (
            tc.tile_pool(name="consts", bufs=1) as consts,
            tc.tile_pool(name="ins", bufs=2 * QS + 1) as ins,
            tc.tile_pool(name="acts", bufs=2) as acts,
            tc.tile_pool(name="outs", bufs=2) as outs,
            tc.tile_pool(name="psa", bufs=2, space="PSUM") as psa,
            tc.tile_pool(name="psv", bufs=2, space="PSUM") as psv,
        ):
            # Warm-up: a no-dep activation so the ~2.7us ACT_TABLE_LOAD for
            # the relu spline tables overlaps the input DMAs instead of
            # sitting on the critical path before the first real relu.
            warm = consts.tile([1, 512], ACT_DT)
            nc.vector.memset(warm[:], 0.0)
            nc.scalar.activation(
                out=warm[0:1, 0:16], in_=warm[0:1, 0:16],
                func=mybir.ActivationFunctionType.Relu,
            )

            # W first, then the first input slot, then the rest: the first
            # fill run needs W + in0; BV (biases) is only needed ~3 runs in.
            W = consts.tile([128, NCONST], ACT_DT)
            nc.sync.dma_start(out=W[:], in_=wpack[:])
            BV = consts.tile([128, NBIAS], mybir.dt.float32)

            in_tiles = {}

            def in_fetch(s):
                if s < NSLOTS and s not in in_tiles:
                    it = ins.tile([128, NB], ACT_DT, tag="in", name=f"in{s}")
                    nc.sync.dma_start(out=it[:], in_=sg[:, s * NB : (s + 1) * NB])
                    in_tiles[s] = it

            def wv(cr, r0=0, r1=128):
                c0, c1 = cr
                return W[r0:r1, c0:c1]

            _pn = [0]

            def psum(eng):
                _pn[0] += 1
                pool = psa if eng == "A" else psv
                return pool.tile(
                    [128, NB], mybir.dt.float32, tag=f"p{eng}", name=f"pp{_pn[0]}"
                )

            def relu_op(eng, out, in_, bcol=None):
                if eng == "A":
                    if bcol is None:
                        nc.scalar.activation(out=out, in_=in_, func=relu)
                    else:
                        nc.scalar.activation(
                            out=out, in_=in_, func=relu,
                            bias=BV[:, bcol : bcol + 1],
                        )
                else:
                    if bcol is None:
                        nc.vector.tensor_scalar_max(out=out, in0=in_, scalar1=0.0)
                    else:
                        nc.vector.tensor_scalar(
                            out=out, in0=in_,
                            scalar1=BV[:, bcol : bcol + 1], scalar2=0.0,
                            op0=add_op, op1=max_op,
                        )

            NETS = ("e", "m", "p")
            L0W = {"e": (C_EW0, 0, 65), "m": (C_MW0, 64, 97), "p": (C_PW0, 0, 97)}
            L1W = {"e": C_EW1, "m": C_MW1, "p": C_PW1}
            L2W = {"e": C_EW2, "m": C_MW2, "p": C_PW2}
            L1B = {"e": B_EB1, "m": B_MB1, "p": B_PB1}
            L2B = {"e": B_EB2, "m": B_MB2, "p": B_PB2}
            outd = {"e": out_e, "m": out_m, "p": out_p}

            a1 = {}  # (net, quad) -> [128, NBQ] tile
            a2 = {}
            _ridx = [0]

            def fill_run(engs, lhsT, rhs_of_slot):
                """QS*2 consecutive same-weight MMs across a quad's slots."""
                pps = []
                for i in range(QS):
                    pp = psum(engs[i])
                    rhs = rhs_of_slot(i)
                    for j in range(0, NB, NMM):
                        nc.tensor.matmul(
                            pp[:, j : j + NMM], lhsT, rhs[:, j : j + NMM],
                            start=True, stop=True,
                        )
                    pps.append(pp)
                return pps

            # 3-stage software pipeline over quads
            for s in range(2):
                in_fetch(s)
            nc.sync.dma_start(out=BV[:], in_=bpack[:])
            for s in range(2, QS + 1):
                in_fetch(s)

            # HAM prewarm: ~2.5us of throwaway matmuls during the DMA
            # lead-in so the PE clock gate is at 8/8 when real fills start.
            wpp = psv.tile([128, NB], mybir.dt.float32, tag="pV", name="warmpp")
            for _ in range(6):
                nc.tensor.matmul(
                    wpp[:, 0:NMM], warm[0:1, 0:128], warm[0:1, 0:NMM],
                    start=True, stop=True,
                )

            for k in range(NQ + 2):
                for s in range(QS):
                    in_fetch((k + 1) * QS + s + 1)

                # stage A: layer-0 for quad k
                if k < NQ:
                    for net in NETS:
                        engs = _run_engines(_ridx[0]); _ridx[0] += 1
                        cr, r0, r1 = L0W[net]
                        pps = fill_run(
                            engs, wv(cr, r0, r1),
                            lambda i: in_tiles[k * QS + i][r0:r1, 0:NB],
                        )
                        t = acts.tile(
                            [128, NBQ], ACT_DT, tag=f"a1{net}", name=f"a1{net}{k}"
                        )
                        for i in range(QS):
                            relu_op(engs[i], t[:, i * NB : (i + 1) * NB], pps[i][:])
                        a1[(net, k)] = t

                # stage B: layer-1 for quad k-1
                q1 = k - 1
                if 0 <= q1 < NQ:
                    for net in NETS:
                        engs = _run_engines(_ridx[0]); _ridx[0] += 1
                        src = a1.pop((net, q1))
                        pps = fill_run(
                            engs, wv(L1W[net]),
                            lambda i: src[:, i * NB : (i + 1) * NB],
                        )
                        t = acts.tile(
                            [128, NBQ], ACT_DT, tag=f"a2{net}", name=f"a2{net}{q1}"
                        )
                        for i in range(QS):
                            relu_op(
                                engs[i], t[:, i * NB : (i + 1) * NB], pps[i][:],
                                L1B[net],
                            )
                        a2[(net, q1)] = t

                # stage C: layer-2 for quad k-2, then DMA out
                q2 = k - 2
                if 0 <= q2 < NQ:
                    for net in NETS:
                        engs = _run_engines(_ridx[0]); _ridx[0] += 1
                        src = a2.pop((net, q2))
                        pps = fill_run(
                            engs, wv(L2W[net]),
                            lambda i: src[:, i * NB : (i + 1) * NB],
                        )
                        t = outs.tile(
                            [128, NBQ], ACT_DT, tag=f"o{net}", name=f"o{net}{q2}"
                        )
                        for i in range(QS):
                            relu_op(
                                engs[i], t[:, i * NB : (i + 1) * NB], pps[i][:],
                                L2B[net],
                            )
                        nc.sync.dma_start(
                            out=outd[net][:, q2 * NBQ : (q2 + 1) * NBQ],
                            in_=t[:],
                        )

    nc.compile()
    return nc


_NC = None
LAST_RESULTS = None  # BassKernelResults from the most recent run (for test.py)


def _get_nc():
    global _NC
    if _NC is None:
        _NC = build_program()
    return _NC


def kernel(**inputs):
    global LAST_RESULTS
    w = {k: np.asarray(v, np.float32) for k, v in inputs.items()}
    s, g = w["s"], w["g"]

    wpack = _pack_consts(w)
    bpack = _pack_biases(w)
    in_maps = []
    for c in range(NCORES):
        r0 = c * BC
        sgT = np.zeros((128, BC), ACT_NP)
        sgT[0:64] = s[r0 : r0 + BC].T.astype(ACT_NP)
        sgT[64] = 1.0
        sgT[65:97] = g[r0 : r0 + BC].T.astype(ACT_NP)
        in_maps.append(
            {"sg": np.ascontiguousarray(sgT), "wpack": wpack, "bpack": bpack}
        )

    nc = _get_nc()
    res = run_bass_kernel_spmd(
        nc,
        in_maps,
        core_ids=list(range(NCORES)),
        trace=bool(int(os.environ.get("KERNEL_TRACE", "0"))),
    )
    LAST_RESULTS = res

    a3e = np.empty((B, 128), np.float32)
    a3m = np.empty((B, 128), np.float32)
    a3p = np.empty((B, 128), np.float32)
    for c in range(NCORES):
        r0 = c * BC
        a3e[r0 : r0 + BC] = res.results[c]["out_e"].T
        a3m[r0 : r0 + BC] = res.results[c]["out_m"].T
        a3p[r0 : r0 + BC] = res.results[c]["out_p"].T

    # ---- host layer-3 projections ----
    z = a3e @ w["ew3"] + w["eb3"]
    mu = a3m @ w["mw3"] + w["mb3"]
    ail = a3p @ w["pw3"] + w["pb3"]

    # ---- host KDE tail + global-norm mix ----
    diff = z[:, None, :] - mu.reshape(B, NG, ADIM)  # [B, 25, 3]
    delta = -0.5 * np.einsum("bnd,bnd->bn", diff, diff) / (H * H)
    p = KDE_C * np.exp(delta)  # [B, 25]
    rho = p.sum(axis=-1)  # [B]
    grad = -np.einsum("bn,bnd->bd", p, diff) / (H * H)
    grad = np.nan_to_num(grad, nan=0.0)
    gnorm = np.linalg.norm(grad)
    gradn = grad / gnorm * NI
    pm = np.tanh(rho * 0.002)[:, None]
    out = pm * ail + (1.0 - pm) * gradn
    return out.astype(np.float32)
